# revision 10
# baseline (speedup 1.0000x reference)
"""Trainium2 Bass kernel for a 2-layer GCN (EnhancedHockeyGNN) — v4.

Strategy (8 NeuronCores, SPMD, ONE NEFF launch):
  - v2's bottleneck was GPSIMD SWDGE descriptor generation for per-edge
    dma_gathers (~7-10 ns/row, serial).  v3 removed layer-1 gathers by
    having the host pre-expand x into per-core edge-slot order (x_perm)
    so layer 1 is a pure sequential stream; aggregation runs in input-
    feature space and W1 is applied once per 128-dst group (W commutes
    with the segment-sum).
  - v4 additionally removes ALL per-edge device arithmetic: the
    symmetric norm is folded as  dinv[src] -> table rows (x_perm rows
    and the xs2 rows are pre-scaled),  dinv[dst] -> one per-group
    [128,128] row-broadcast multiply before BN.  One-hot matrices are
    then pure 0/1, precomputed on the host in fp8 and streamed as
    sequential DMA; the PE consumes them directly (fp16 x fp8 matmul).
  - Layer 2 still gathers xs2 rows (device-computed); the dma_gathers
    are spread across 4 SWDGE queues so descriptor generation overlaps.
  - AllGather of the xs2 table runs in 4 chunks during layer 1.
  - Readout computes log-softmax for every padded node; the host
    selects the requested game_indices rows.
"""
import math
import os

import numpy as np

# ---------------------------------------------------------------- constants
N = 100000
F_IN = 128
H = 128
NC = 8
SHARD = 12544            # multiple of 128; 8 * 12544 = 100352 >= N
NPAD = NC * SHARD
NQ = 4                   # AllGather chunks / int16 gather sub-tables
GROUP_EDGES = 2048
GROUP_DSTS = 128
WAVE = 8                 # groups per wave; L1 streams half-waves of 4
EPS = 1e-5
GQ_MAX = 32              # NC * GQ_MAX * 128 == 32768 (int16 reach)

_CACHE = {}


def _chunks(n, k):
    k = min(k, n)
    base, rem = n // k, n % k
    out, lo = [], 0
    for i in range(k):
        hi = lo + base + (1 if i < rem else 0)
        out.append((lo, hi))
        lo = hi
    return out


# ---------------------------------------------------------------- host prep
def _bin_pack(counts, G):
    order = np.argsort(-counts, kind="stable")
    bin_edges = np.zeros(G, dtype=np.int64)
    bin_nodes = np.zeros(G, dtype=np.int64)
    group_of = np.full(counts.shape[0], -1, dtype=np.int32)
    pos_in_group = np.full(counts.shape[0], -1, dtype=np.int32)
    for d in order:
        c = counts[d]
        placed = False
        for b in range(G):
            if bin_edges[b] + c <= GROUP_EDGES and bin_nodes[b] < GROUP_DSTS:
                group_of[d] = b
                pos_in_group[d] = bin_nodes[b]
                bin_edges[b] += c
                bin_nodes[b] += 1
                placed = True
                break
        if not placed:
            return None
    return group_of, pos_in_group


def _onehot_fp8(dloc):
    """dloc: [S] float (0..127 or 300=pad) -> [128, S] fp16 0/1 one-hot.

    Tile-major: out[p, t*128 + j] = (dloc[t*128+p] == j).
    """
    S = dloc.shape[0]
    nt = S // 128
    out = np.zeros((nt, 128, 128), dtype=np.float16)
    d = dloc.reshape(nt, 128).astype(np.int32)
    t_i, p_i = np.nonzero((d >= 0) & (d < 128))
    out[t_i, p_i, d[t_i, p_i]] = 1.0
    return np.ascontiguousarray(out.transpose(1, 0, 2)).reshape(128, S)


def _prepare(edge_index):
    src = np.asarray(edge_index[0], dtype=np.int64)
    dst = np.asarray(edge_index[1], dtype=np.int64)
    deg = np.bincount(dst, minlength=N).astype(np.float64) + 1.0
    dinv = 1.0 / np.sqrt(deg)
    dinv_pad = np.zeros(NPAD, dtype=np.float64)
    dinv_pad[:N] = dinv

    sall = np.concatenate([src, np.arange(N, dtype=np.int64)])
    dall = np.concatenate([dst, np.arange(N, dtype=np.int64)])
    owner = dall // SHARD

    # ----- bin packing per core, shared global G
    Es = [int((owner == c).sum()) for c in range(NC)]
    G = max(int(math.ceil(e / GROUP_EDGES)) for e in Es)
    while True:
        packs = []
        ok = True
        for c in range(NC):
            m = owner == c
            d0 = (dall[m] - c * SHARD).astype(np.int64)
            counts = np.bincount(d0, minlength=SHARD)
            r = _bin_pack(counts, G)
            if r is None:
                ok = False
                break
            packs.append((r[0].astype(np.int64), r[1].astype(np.int64),
                          d0, sall[m]))
        if ok:
            break
        G += 1

    NW = (G + WAVE - 1) // WAVE
    ch_d = _chunks(G, NQ)
    Gq = [hi - lo for lo, hi in ch_d]
    assert max(Gq) <= GQ_MAX, (G, Gq)
    chunk_of_g = np.zeros(G, dtype=np.int64)
    lo_of_chunk = np.array([lo for lo, _ in ch_d], dtype=np.int64)
    for q, (lo, hi) in enumerate(ch_d):
        chunk_of_g[lo:hi] = q

    # ----- per-node location in the xs2 (layer-2) table
    node_g2 = np.zeros(NPAD, dtype=np.int64)
    node_pos2 = np.zeros(NPAD, dtype=np.int64)
    for c in range(NC):
        node_g2[c * SHARD:(c + 1) * SHARD] = packs[c][0]
        node_pos2[c * SHARD:(c + 1) * SHARD] = packs[c][1]
    node_o = np.arange(NPAD, dtype=np.int64) // SHARD
    node_q2 = chunk_of_g[node_g2]
    gq_arr = np.array(Gq, dtype=np.int64)
    node_idx2 = (node_o * gq_arr[node_q2] * 128
                 + (node_g2 - lo_of_chunk[node_q2]) * 128 + node_pos2)

    wave_of_g = np.arange(G, dtype=np.int64) // WAVE

    # ----- per-core dinv layouts (dst side)
    dr_list, dc_list = [], []
    for c in range(NC):
        group_of, pos, _, _ = packs[c]
        v = np.zeros(G * 128, dtype=np.float64)
        v[group_of * 128 + pos] = dinv_pad[c * SHARD:(c + 1) * SHARD]
        dr = np.broadcast_to(v.astype(np.float16), (128, G * 128)).copy()
        dc = v.reshape(G, 128).T.astype(np.float32).copy()
        dr_list.append(dr)
        dc_list.append(dc)

    # ----- layer 1: slots ordered by group, padded per group to x128
    tiles_all = []
    for c in range(NC):
        group_of, pos, d0, s_nodes = packs[c]
        cnt_c = np.bincount(group_of[d0], minlength=G)
        tiles_all.append(np.maximum(1, -(-cnt_c // 128)))
    tiles_g = np.stack(tiles_all).max(axis=0)
    nt1 = int(tiles_g.sum())
    starts_t = np.concatenate([[0], np.cumsum(tiles_g)[:-1]])
    per_core = []
    for c in range(NC):
        group_of, pos, d0, s_nodes = packs[c]
        e_g = group_of[d0]
        order = np.argsort(e_g, kind="stable")
        e_g_s = e_g[order]
        e_src = s_nodes[order]
        e_dloc = pos[d0][order]
        cnt = np.bincount(e_g_s, minlength=G)
        cnt_cum = np.concatenate([[0], np.cumsum(cnt)[:-1]])
        slot = (starts_t[e_g_s] * 128
                + (np.arange(len(e_g_s)) - cnt_cum[e_g_s]))
        S1 = nt1 * 128
        sl_src = np.full(S1, -1, dtype=np.int64)
        sl_dloc = np.full(S1, 300.0, dtype=np.float32)
        sl_src[slot] = e_src
        sl_dloc[slot] = e_dloc
        per_core.append(dict(oh1=_onehot_fp8(sl_dloc), xp_idx=sl_src,
                             dinvrow=dr_list[c], dinvcol=dc_list[c]))

    # ----- layer 2: (wave, quarter) gather calls
    KSZ = NW * NQ * G

    def seg_key(e_g, e_q):
        return (wave_of_g[e_g] * NQ + e_q) * G + e_g

    edges2 = []
    for c in range(NC):
        group_of, pos, d0, s_nodes = packs[c]
        e_g = group_of[d0]
        e_dloc = pos[d0].astype(np.float32)
        e_q2 = node_q2[s_nodes]
        e_i2 = node_idx2[s_nodes].astype(np.int16)
        edges2.append((e_g, e_dloc, e_q2, e_i2))

    cnts = np.zeros((NC, KSZ), dtype=np.int64)
    for c in range(NC):
        e = edges2[c]
        cnts[c] = np.bincount(seg_key(e[0], e[2]), minlength=KSZ)
    tseg = -(-cnts.max(axis=0) // 128)
    seg_off = np.zeros(KSZ, dtype=np.int64)
    calls2 = []
    k = 0
    for w in range(NW):
        wcalls = []
        g_lo, g_hi = w * WAVE, min((w + 1) * WAVE, G)
        for q in range(NQ):
            k0 = k
            segs = []
            for g in range(g_lo, g_hi):
                key = (w * NQ + q) * G + g
                t = int(tseg[key])
                if t == 0:
                    continue
                seg_off[key] = k * 128
                segs.append((g, t))
                k += t
            if segs:
                wcalls.append((q, k0, segs))
        calls2.append(wcalls)
    nt2 = k
    tmax2 = 1
    for wcalls in calls2:
        for q, k0, segs in wcalls:
            tmax2 = max(tmax2, sum(t for _, t in segs))

    for c in range(NC):
        e = edges2[c]
        key = seg_key(e[0], e[2])
        order = np.argsort(key, kind="stable")
        ks = key[order]
        first = np.searchsorted(ks, ks, side="left")
        dest = seg_off[ks] + (np.arange(len(ks)) - first)
        slots_i = np.zeros(nt2 * 128, dtype=np.int16)
        slots_dloc = np.full(nt2 * 128, 300.0, dtype=np.float32)
        slots_i[dest] = e[3][order]
        slots_dloc[dest] = e[1][order]
        idx2 = np.zeros((128, nt2 * 8), dtype=np.int16)
        for wcalls in calls2:
            for q, k0, segs in wcalls:
                tcall = sum(t for _, t in segs)
                arr = slots_i[k0 * 128:(k0 + tcall) * 128]
                idx2[0:16, k0 * 8:(k0 + tcall) * 8] = arr.reshape(-1, 16).T
        for r in range(1, 8):
            idx2[16 * r:16 * (r + 1)] = idx2[0:16]
        per_core[c].update(oh2=_onehot_fp8(slots_dloc), idx2=idx2)

    pad_cji = np.zeros((NPAD, 3), dtype=np.int64)
    pad_cji[:, 0] = node_o
    pad_cji[:, 1] = node_g2
    pad_cji[:, 2] = node_pos2

    structure = dict(G=G, NW=NW, ch_d=ch_d, Gq=Gq, calls2=calls2,
                     tiles_g=tiles_g.tolist(), nt1=nt1, nt2=nt2,
                     tmax2=tmax2)
    return per_core, structure, pad_cji, dinv


def _fold_bn(gamma, beta, mean, var, b):
    s = (gamma / np.sqrt(var + EPS)).astype(np.float32)
    t = ((b - mean) * s + beta).astype(np.float32)
    return s.reshape(H, 1), t.reshape(H, 1)


# ---------------------------------------------------------------- bass build
def _build(st_):
    import concourse.bacc as bacc
    import concourse.bass as bass
    import concourse.mybir as mybir
    import concourse.tile as tile

    fp32 = mybir.dt.float32
    fp16 = mybir.dt.float16
    fp8 = mybir.dt.float8e4
    i16 = mybir.dt.int16
    AF = mybir.ActivationFunctionType
    AL = mybir.AluOpType

    G = st_["G"]
    ch_d = st_["ch_d"]
    Gq = st_["Gq"]
    calls2 = st_["calls2"]
    tiles_g = st_["tiles_g"]
    nt1, nt2, TMAX2 = st_["nt1"], st_["nt2"], st_["tmax2"]
    chunk_end = {hi - 1: q for q, (lo, hi) in enumerate(ch_d)}
    g_chunk = {}
    for q, (lo, hi) in enumerate(ch_d):
        for g in range(lo, hi):
            g_chunk[g] = (q, lo)
    starts_t = [0]
    for g in range(1, G):
        starts_t.append(starts_t[-1] + tiles_g[g - 1])
    n_queues = int(os.environ.get("K_QUEUES", "4"))

    nc = bacc.Bacc(None, target_bir_lowering=False, debug=False,
                   num_devices=NC, num_swdge_queues=max(1, n_queues))

    xp_in = nc.dram_tensor("xperm", [128, nt1 * 128], fp16,
                           kind="ExternalInput")
    oh1_in = nc.dram_tensor("oh1", [128, nt1 * 128], fp16,
                            kind="ExternalInput")
    oh2_in = nc.dram_tensor("oh2", [128, nt2 * 128], fp16,
                            kind="ExternalInput")
    w1_in = nc.dram_tensor("W1", [F_IN, H], fp16, kind="ExternalInput")
    w2_in = nc.dram_tensor("W2", [H, H], fp16, kind="ExternalInput")
    wf_in = nc.dram_tensor("Wf", [H, 2], fp16, kind="ExternalInput")
    s1_in = nc.dram_tensor("s1", [H, 1], fp32, kind="ExternalInput")
    t1_in = nc.dram_tensor("t1", [H, 1], fp32, kind="ExternalInput")
    s2_in = nc.dram_tensor("s2", [H, 1], fp32, kind="ExternalInput")
    t2_in = nc.dram_tensor("t2", [H, 1], fp32, kind="ExternalInput")
    bf_in = nc.dram_tensor("bf_rep", [128, 2], fp32, kind="ExternalInput")
    dr_in = nc.dram_tensor("dinvrow", [128, G * 128], fp16,
                           kind="ExternalInput")
    dc_in = nc.dram_tensor("dinvcol", [128, G], fp32, kind="ExternalInput")
    idx2_in = nc.dram_tensor("idx2", [128, nt2 * 8], i16,
                             kind="ExternalInput")
    out_lp = nc.dram_tensor("logp", [128, 2 * G], fp32,
                            kind="ExternalOutput")

    with tile.TileContext(nc) as tc:
        with (
            tc.tile_pool(name="res", bufs=1) as res,
            tc.tile_pool(name="st", bufs=1) as st,
            tc.tile_pool(name="ps", bufs=1, space="PSUM") as ps,
            tc.tile_pool(name="dram", bufs=1, space="DRAM") as dram,
        ):
            w1_t = res.tile([F_IN, H], fp16)
            w2_t = res.tile([H, H], fp16)
            wf_t = res.tile([H, 2], fp16)
            s1_t = res.tile([H, 1], fp32)
            t1_t = res.tile([H, 1], fp32)
            s2_t = res.tile([H, 1], fp32)
            t2_t = res.tile([H, 1], fp32)
            bf_t = res.tile([128, 2], fp32)
            dr_t = res.tile([128, G * 128], fp16)
            dc_t = res.tile([128, G], fp32)
            idx2_t = res.tile([128, nt2 * 8], i16)
            for t_, i_ in ((w1_t, w1_in), (w2_t, w2_in), (wf_t, wf_in),
                           (s1_t, s1_in), (t1_t, t1_in), (s2_t, s2_in),
                           (t2_t, t2_in), (bf_t, bf_in), (dr_t, dr_in),
                           (dc_t, dc_in), (idx2_t, idx2_in)):
                nc.sync.dma_start(out=t_[:], in_=i_[:])

            xs2_shard = [dram.tile([Gq[q] * 128, H], fp16,
                                   name=f"xs2_shard{q}") for q in range(NQ)]
            xs2_full = [dram.tile([NC * Gq[q] * 128, H], fp16,
                                  name=f"xs2_full{q}") for q in range(NQ)]

            lg = res.tile([128, 2 * G], fp32)
            nc.vector.memset(lg[:], 0.0)

            # =================== layer 1: streamed, zero gathers ==========
            def l1_epilogue(g, pre_ap):
                pre_sb = st.tile([128, 128], fp16, name="pre", tag="pre",
                                 bufs=4)
                nc.scalar.copy(out=pre_sb[:], in_=pre_ap)
                h_ps = ps.tile([128, 512], fp32, name="hp", tag="hp", bufs=2)
                nc.tensor.matmul(h_ps[:, :H], w1_t[:], pre_sb[:],
                                 start=True, stop=True)
                hs = st.tile([128, 128], fp16, name="hs", tag="hs", bufs=4)
                nc.vector.tensor_tensor(
                    out=hs[:], in0=h_ps[:, :H],
                    in1=dr_t[:, g * 128:(g + 1) * 128], op=AL.mult)
                hT = st.tile([128, 128], fp16, name="hT", tag="hT", bufs=4)
                nc.scalar.activation(out=hT[:], in_=hs[:], func=AF.Relu,
                                     bias=t1_t[:], scale=s1_t[:])
                x2_ps = ps.tile([128, 512], fp32, name="x2p", tag="x2p",
                                bufs=2)
                nc.tensor.matmul(x2_ps[:, :H], hT[:], w2_t[:],
                                 start=True, stop=True)
                x2_sb = st.tile([128, 128], fp16, name="x2", tag="x2",
                                bufs=4)
                nc.scalar.activation(out=x2_sb[:], in_=x2_ps[:, :H],
                                     func=AF.Copy, bias=0.0,
                                     scale=dc_t[:, g:g + 1])
                q, lo = g_chunk[g]
                nc.sync.dma_start(
                    out=xs2_shard[q][(g - lo) * 128:(g - lo + 1) * 128, :],
                    in_=x2_sb[:])
                if g in chunk_end:
                    qq = chunk_end[g]
                    nc.gpsimd.collective_compute(
                        "AllGather", mybir.AluOpType.bypass,
                        replica_groups=[list(range(NC))],
                        ins=[xs2_shard[qq][:].opt()],
                        outs=[xs2_full[qq][:].opt()],
                    )

            HW = 4  # groups per L1 half-wave (one PSUM bank)
            NHW = (G + HW - 1) // HW
            for hw in range(NHW):
                g_lo, g_hi = hw * HW, min((hw + 1) * HW, G)
                t_lo = starts_t[g_lo]
                t_hi = starts_t[g_hi - 1] + tiles_g[g_hi - 1]
                Tw = t_hi - t_lo
                xpw = st.tile([128, Tw, 128], fp16, name="xpw", tag="xpw",
                              bufs=2)
                nc.sync.dma_start(out=xpw[:],
                                  in_=xp_in[:, t_lo * 128:t_hi * 128])
                ohw = st.tile([128, Tw, 128], fp16, name="ohw", tag="ohw",
                              bufs=2)
                nc.sync.dma_start(out=ohw[:],
                                  in_=oh1_in[:, t_lo * 128:t_hi * 128])
                bank = ps.tile([128, 512], fp32, name="pg", tag="pg", bufs=4)
                tmax_hw = max(tiles_g[g] for g in range(g_lo, g_hi))
                for t in range(tmax_hw):
                    for g in range(g_lo, g_hi):
                        tg = tiles_g[g]
                        if t >= tg:
                            continue
                        col = (g - g_lo) * 128
                        k = starts_t[g] + t - t_lo
                        nc.tensor.matmul(
                            bank[:, col:col + 128], xpw[:, k, :],
                            ohw[:, k, :],
                            start=(t == 0), stop=(t == tg - 1),
                            skip_group_check=True)
                for g in range(g_lo, g_hi):
                    col = (g - g_lo) * 128
                    l1_epilogue(g, bank[:, col:col + 128])

            # =================== layer 2: gathered from xs2_full ==========
            for w, wcalls in enumerate(calls2):
                remaining = {}
                for q, k0, segs in wcalls:
                    for g, tg in segs:
                        remaining[g] = remaining.get(g, 0) + tg
                glist = sorted(remaining)
                g_lo = w * WAVE
                banks2 = [ps.tile([128, 512], fp32, name="pg2", tag="pg",
                                  bufs=4) for _ in range(2)]
                started = set()

                def pg2_ap(g):
                    b = banks2[(g - g_lo) // 4]
                    col = ((g - g_lo) % 4) * 128
                    return b[:, col:col + 128]

                for q, k0, segs in wcalls:
                    tcall = sum(tg for _, tg in segs)
                    ni = tcall * 128
                    msg = st.tile([128, TMAX2, 128], fp16, name="msg",
                                  tag="msg", bufs=3)
                    src_ap = xs2_full[q][:]
                    nc.gpsimd.dma_gather(
                        msg[:, :tcall, :], src_ap,
                        idx2_t[:, k0 * 8:(k0 + tcall) * 8],
                        ni, ni, H, elem_step=src_ap.ap[0][0],
                        single_packet=False,
                        queue_num=(q % n_queues) if n_queues > 1 else 0)
                    oh2w = st.tile([128, TMAX2, 128], fp16, name="oh2w",
                                   tag="oh2w", bufs=2)
                    nc.sync.dma_start(
                        out=oh2w[:, :tcall, :],
                        in_=oh2_in[:, k0 * 128:(k0 + tcall) * 128])
                    order = []
                    tl = 0
                    for g, tg in segs:
                        for j in range(tg):
                            order.append((j, g, tl))
                            tl += 1
                    order.sort(key=lambda x: (x[0], x[1]))
                    for _, g, tl in order:
                        first = g not in started
                        started.add(g)
                        nc.tensor.matmul(
                            pg2_ap(g), msg[:, tl, :], oh2w[:, tl, :],
                            start=first, stop=(remaining[g] == 1),
                            skip_group_check=True)
                        remaining[g] -= 1
                for g in glist:
                    hs2 = st.tile([128, 128], fp16, name="hs2", tag="hs",
                                  bufs=4)
                    nc.vector.tensor_tensor(
                        out=hs2[:], in0=pg2_ap(g),
                        in1=dr_t[:, g * 128:(g + 1) * 128], op=AL.mult)
                    hT2 = st.tile([128, 128], fp16, name="hT2", tag="hT",
                                  bufs=4)
                    nc.scalar.activation(out=hT2[:], in_=hs2[:],
                                         func=AF.Relu, bias=t2_t[:],
                                         scale=s2_t[:])
                    plg = ps.tile([128, 512], fp32, name="plg", tag="hp",
                                  bufs=2)
                    nc.tensor.matmul(plg[:, 0:2], hT2[:], wf_t[:],
                                     start=True, stop=True)
                    nc.vector.tensor_add(out=lg[:, 2 * g:2 * g + 2],
                                         in0=plg[:, 0:2], in1=bf_t[:])

            # =================== log-softmax over the 2 logits ============
            def strided(base, start):
                a = base[:]
                return bass.AP(a.tensor, a.offset + start, [a.ap[0], [2, G]])

            z0, z1 = strided(lg, 0), strided(lg, 1)
            mx = res.tile([128, G], fp32)
            nc.vector.tensor_tensor(out=mx[:], in0=z0, in1=z1, op=AL.max)
            sm0 = res.tile([128, G], fp32)
            sm1 = res.tile([128, G], fp32)
            nc.vector.tensor_sub(out=sm0[:], in0=z0, in1=mx[:])
            nc.vector.tensor_sub(out=sm1[:], in0=z1, in1=mx[:])
            e0 = res.tile([128, G], fp32)
            e1 = res.tile([128, G], fp32)
            nc.scalar.activation(out=e0[:], in_=sm0[:], func=AF.Exp)
            nc.scalar.activation(out=e1[:], in_=sm1[:], func=AF.Exp)
            se = res.tile([128, G], fp32)
            nc.vector.tensor_add(out=se[:], in0=e0[:], in1=e1[:])
            ls = res.tile([128, G], fp32)
            nc.scalar.activation(out=ls[:], in_=se[:], func=AF.Ln)
            nc.vector.tensor_sub(out=sm0[:], in0=sm0[:], in1=ls[:])
            nc.vector.tensor_sub(out=sm1[:], in0=sm1[:], in1=ls[:])
            lpo = res.tile([128, 2 * G], fp32)
            nc.vector.tensor_copy(out=strided(lpo, 0), in_=sm0[:])
            nc.vector.tensor_copy(out=strided(lpo, 1), in_=sm1[:])
            nc.sync.dma_start(out=out_lp[:], in_=lpo[:])

    nc.compile()
    return nc


# ---------------------------------------------------------------- main entry
def _run(x, edge_index, game_indices,
         W1, b1, g1, be1, m1, v1, W2, b2, g2, be2, m2, v2, Wf, bf,
         trace=False):
    from concourse import bass_utils

    ei = np.asarray(edge_index)
    key = ("prep", int(ei[0, 0]), int(ei.sum() % (1 << 31)))
    if key in _CACHE:
        per_core, structure, pad_cji, dinv = _CACHE[key]
    else:
        per_core, structure, pad_cji, dinv = _prepare(ei)
        _CACHE.clear()
        _CACHE[key] = (per_core, structure, pad_cji, dinv)

    skey = ("bass", structure["G"], structure["nt1"], structure["nt2"],
            structure["tmax2"])
    if skey in _CACHE:
        nc = _CACHE[skey]
    else:
        nc = _build(structure)
        _CACHE[skey] = nc

    nt1 = structure["nt1"]

    # xs = x * dinv[src]: the src-side norm folded into the table rows
    xs = (np.asarray(x, dtype=np.float32)
          * dinv.astype(np.float32)[:, None]).astype(np.float16)
    s1, t1 = _fold_bn(np.asarray(g1), np.asarray(be1), np.asarray(m1),
                      np.asarray(v1), np.asarray(b1))
    s2, t2 = _fold_bn(np.asarray(g2), np.asarray(be2), np.asarray(m2),
                      np.asarray(v2), np.asarray(b2))
    bf_rep = np.broadcast_to(np.asarray(bf, dtype=np.float32),
                             (128, 2)).copy()
    w1h = np.asarray(W1, np.float16)
    w2h = np.asarray(W2, np.float16)
    wfh = np.asarray(Wf, np.float16)

    in_maps = []
    for c in range(NC):
        pc = per_core[c]
        xp_idx = pc["xp_idx"]
        xp = xs[np.maximum(xp_idx, 0)]
        xp[xp_idx < 0] = 0
        xpt = np.ascontiguousarray(
            xp.reshape(nt1, 128, F_IN).transpose(1, 0, 2)
        ).reshape(128, nt1 * F_IN)
        in_maps.append(dict(
            xperm=xpt,
            oh1=pc["oh1"], oh2=pc["oh2"],
            W1=w1h, W2=w2h, Wf=wfh, s1=s1, t1=t1, s2=s2, t2=t2,
            bf_rep=bf_rep, dinvrow=pc["dinvrow"], dinvcol=pc["dinvcol"],
            idx2=pc["idx2"],
        ))
    res = bass_utils.run_bass_kernel_spmd(
        nc, in_maps, core_ids=list(range(NC)), trace=trace)

    gi = np.asarray(game_indices, dtype=np.int64)
    cji = pad_cji[gi]
    lp = np.stack([res.results[c]["logp"] for c in range(NC)])
    out = np.empty((gi.shape[0], 2), dtype=np.float32)
    out[:, 0] = lp[cji[:, 0], cji[:, 2], 2 * cji[:, 1]]
    out[:, 1] = lp[cji[:, 0], cji[:, 2], 2 * cji[:, 1] + 1]
    return out, res


def kernel(**inputs):
    out, _ = _run(**inputs)
    return out


def kernel_profiled(**inputs):
    out, res = _run(**inputs, trace=True)
    return out, res


# revision 11
# speedup vs baseline: 1.0363x; 1.0363x over previous
"""Trainium2 Bass kernel for a 2-layer GCN (EnhancedHockeyGNN) — v4.

Strategy (8 NeuronCores, SPMD, ONE NEFF launch):
  - v2's bottleneck was GPSIMD SWDGE descriptor generation for per-edge
    dma_gathers (~7-10 ns/row, serial).  v3 removed layer-1 gathers by
    having the host pre-expand x into per-core edge-slot order (x_perm)
    so layer 1 is a pure sequential stream; aggregation runs in input-
    feature space and W1 is applied once per 128-dst group (W commutes
    with the segment-sum).
  - v4 additionally removes ALL per-edge device arithmetic: the
    symmetric norm is folded as  dinv[src] -> table rows (x_perm rows
    and the xs2 rows are pre-scaled),  dinv[dst] -> one per-group
    [128,128] row-broadcast multiply before BN.  One-hot matrices are
    then pure 0/1, precomputed on the host in fp8 and streamed as
    sequential DMA; the PE consumes them directly (fp16 x fp8 matmul).
  - Layer 2 still gathers xs2 rows (device-computed); the dma_gathers
    are spread across 4 SWDGE queues so descriptor generation overlaps.
  - AllGather of the xs2 table runs in 4 chunks during layer 1.
  - Readout computes log-softmax for every padded node; the host
    selects the requested game_indices rows.
"""
import math
import os

import numpy as np

# ---------------------------------------------------------------- constants
N = 100000
F_IN = 128
H = 128
NC = 8
SHARD = 12544            # multiple of 128; 8 * 12544 = 100352 >= N
NPAD = NC * SHARD
NQ = 4                   # AllGather chunks / int16 gather sub-tables
GROUP_EDGES = 2048
GROUP_DSTS = 128
WAVE = 8                 # groups per wave; L1 streams half-waves of 4
EPS = 1e-5
GQ_MAX = 32              # NC * GQ_MAX * 128 == 32768 (int16 reach)

_CACHE = {}


def _chunks(n, k):
    k = min(k, n)
    base, rem = n // k, n % k
    out, lo = [], 0
    for i in range(k):
        hi = lo + base + (1 if i < rem else 0)
        out.append((lo, hi))
        lo = hi
    return out


# ---------------------------------------------------------------- host prep
def _bin_pack(counts, G):
    order = np.argsort(-counts, kind="stable")
    bin_edges = np.zeros(G, dtype=np.int64)
    bin_nodes = np.zeros(G, dtype=np.int64)
    group_of = np.full(counts.shape[0], -1, dtype=np.int32)
    pos_in_group = np.full(counts.shape[0], -1, dtype=np.int32)
    for d in order:
        c = counts[d]
        placed = False
        for b in range(G):
            if bin_edges[b] + c <= GROUP_EDGES and bin_nodes[b] < GROUP_DSTS:
                group_of[d] = b
                pos_in_group[d] = bin_nodes[b]
                bin_edges[b] += c
                bin_nodes[b] += 1
                placed = True
                break
        if not placed:
            return None
    return group_of, pos_in_group


def _onehot_fp8(dloc):
    """dloc: [S] float (0..127 or 300=pad) -> [128, S] fp8e4-bit uint8.

    Tile-major: out[p, t*128 + j] = (dloc[t*128+p] == j) ? 0x38 : 0
    (0x38 is 1.0 in fp8e4m3).
    """
    S = dloc.shape[0]
    nt = S // 128
    out = np.zeros((nt, 128, 128), dtype=np.uint8)
    d = dloc.reshape(nt, 128).astype(np.int32)
    t_i, p_i = np.nonzero((d >= 0) & (d < 128))
    out[t_i, p_i, d[t_i, p_i]] = 0x38
    return np.ascontiguousarray(out.transpose(1, 0, 2)).reshape(128, S)


def _prepare(edge_index):
    src = np.asarray(edge_index[0], dtype=np.int64)
    dst = np.asarray(edge_index[1], dtype=np.int64)
    deg = np.bincount(dst, minlength=N).astype(np.float64) + 1.0
    dinv = 1.0 / np.sqrt(deg)
    dinv_pad = np.zeros(NPAD, dtype=np.float64)
    dinv_pad[:N] = dinv

    sall = np.concatenate([src, np.arange(N, dtype=np.int64)])
    dall = np.concatenate([dst, np.arange(N, dtype=np.int64)])
    owner = dall // SHARD

    # ----- bin packing per core, shared global G
    Es = [int((owner == c).sum()) for c in range(NC)]
    G = max(int(math.ceil(e / GROUP_EDGES)) for e in Es)
    while True:
        packs = []
        ok = True
        for c in range(NC):
            m = owner == c
            d0 = (dall[m] - c * SHARD).astype(np.int64)
            counts = np.bincount(d0, minlength=SHARD)
            r = _bin_pack(counts, G)
            if r is None:
                ok = False
                break
            packs.append((r[0].astype(np.int64), r[1].astype(np.int64),
                          d0, sall[m]))
        if ok:
            break
        G += 1

    NW = (G + WAVE - 1) // WAVE
    ch_d = _chunks(G, NQ)
    Gq = [hi - lo for lo, hi in ch_d]
    assert max(Gq) <= GQ_MAX, (G, Gq)
    chunk_of_g = np.zeros(G, dtype=np.int64)
    lo_of_chunk = np.array([lo for lo, _ in ch_d], dtype=np.int64)
    for q, (lo, hi) in enumerate(ch_d):
        chunk_of_g[lo:hi] = q

    # ----- per-node location in the xs2 (layer-2) table
    node_g2 = np.zeros(NPAD, dtype=np.int64)
    node_pos2 = np.zeros(NPAD, dtype=np.int64)
    for c in range(NC):
        node_g2[c * SHARD:(c + 1) * SHARD] = packs[c][0]
        node_pos2[c * SHARD:(c + 1) * SHARD] = packs[c][1]
    node_o = np.arange(NPAD, dtype=np.int64) // SHARD
    node_q2 = chunk_of_g[node_g2]
    gq_arr = np.array(Gq, dtype=np.int64)
    node_idx2 = (node_o * gq_arr[node_q2] * 128
                 + (node_g2 - lo_of_chunk[node_q2]) * 128 + node_pos2)

    wave_of_g = np.arange(G, dtype=np.int64) // WAVE

    # ----- per-core dinv layouts (dst side)
    dr_list, dc_list = [], []
    for c in range(NC):
        group_of, pos, _, _ = packs[c]
        v = np.zeros(G * 128, dtype=np.float64)
        v[group_of * 128 + pos] = dinv_pad[c * SHARD:(c + 1) * SHARD]
        dr = np.broadcast_to(v.astype(np.float16), (128, G * 128)).copy()
        dc = v.reshape(G, 128).T.astype(np.float32).copy()
        dr_list.append(dr)
        dc_list.append(dc)

    # ----- layer 1: slots ordered by group, padded per group to x128
    tiles_all = []
    for c in range(NC):
        group_of, pos, d0, s_nodes = packs[c]
        cnt_c = np.bincount(group_of[d0], minlength=G)
        tiles_all.append(np.maximum(1, -(-cnt_c // 128)))
    tiles_g = np.stack(tiles_all).max(axis=0)
    nt1 = int(tiles_g.sum())
    starts_t = np.concatenate([[0], np.cumsum(tiles_g)[:-1]])
    per_core = []
    for c in range(NC):
        group_of, pos, d0, s_nodes = packs[c]
        e_g = group_of[d0]
        order = np.argsort(e_g, kind="stable")
        e_g_s = e_g[order]
        e_src = s_nodes[order]
        e_dloc = pos[d0][order]
        cnt = np.bincount(e_g_s, minlength=G)
        cnt_cum = np.concatenate([[0], np.cumsum(cnt)[:-1]])
        slot = (starts_t[e_g_s] * 128
                + (np.arange(len(e_g_s)) - cnt_cum[e_g_s]))
        S1 = nt1 * 128
        sl_src = np.full(S1, -1, dtype=np.int64)
        sl_dloc = np.full(S1, 300.0, dtype=np.float32)
        sl_src[slot] = e_src
        sl_dloc[slot] = e_dloc
        per_core.append(dict(oh1=_onehot_fp8(sl_dloc), xp_idx=sl_src,
                             dinvrow=dr_list[c], dinvcol=dc_list[c]))

    # ----- layer 2: (wave, quarter) gather calls
    KSZ = NW * NQ * G

    def seg_key(e_g, e_q):
        return (wave_of_g[e_g] * NQ + e_q) * G + e_g

    edges2 = []
    for c in range(NC):
        group_of, pos, d0, s_nodes = packs[c]
        e_g = group_of[d0]
        e_dloc = pos[d0].astype(np.float32)
        e_q2 = node_q2[s_nodes]
        e_i2 = node_idx2[s_nodes].astype(np.int16)
        edges2.append((e_g, e_dloc, e_q2, e_i2))

    cnts = np.zeros((NC, KSZ), dtype=np.int64)
    for c in range(NC):
        e = edges2[c]
        cnts[c] = np.bincount(seg_key(e[0], e[2]), minlength=KSZ)
    tseg = -(-cnts.max(axis=0) // 128)
    seg_off = np.zeros(KSZ, dtype=np.int64)
    calls2 = []
    k = 0
    for w in range(NW):
        wcalls = []
        g_lo, g_hi = w * WAVE, min((w + 1) * WAVE, G)
        for q in range(NQ):
            k0 = k
            segs = []
            for g in range(g_lo, g_hi):
                key = (w * NQ + q) * G + g
                t = int(tseg[key])
                if t == 0:
                    continue
                seg_off[key] = k * 128
                segs.append((g, t))
                k += t
            if segs:
                wcalls.append((q, k0, segs))
        calls2.append(wcalls)
    nt2 = k
    tmax2 = 1
    for wcalls in calls2:
        for q, k0, segs in wcalls:
            tmax2 = max(tmax2, sum(t for _, t in segs))

    for c in range(NC):
        e = edges2[c]
        key = seg_key(e[0], e[2])
        order = np.argsort(key, kind="stable")
        ks = key[order]
        first = np.searchsorted(ks, ks, side="left")
        dest = seg_off[ks] + (np.arange(len(ks)) - first)
        slots_i = np.zeros(nt2 * 128, dtype=np.int16)
        slots_dloc = np.full(nt2 * 128, 300.0, dtype=np.float32)
        slots_i[dest] = e[3][order]
        slots_dloc[dest] = e[1][order]
        idx2 = np.zeros((128, nt2 * 8), dtype=np.int16)
        for wcalls in calls2:
            for q, k0, segs in wcalls:
                tcall = sum(t for _, t in segs)
                arr = slots_i[k0 * 128:(k0 + tcall) * 128]
                idx2[0:16, k0 * 8:(k0 + tcall) * 8] = arr.reshape(-1, 16).T
        for r in range(1, 8):
            idx2[16 * r:16 * (r + 1)] = idx2[0:16]
        per_core[c].update(oh2=_onehot_fp8(slots_dloc), idx2=idx2)

    pad_cji = np.zeros((NPAD, 3), dtype=np.int64)
    pad_cji[:, 0] = node_o
    pad_cji[:, 1] = node_g2
    pad_cji[:, 2] = node_pos2

    structure = dict(G=G, NW=NW, ch_d=ch_d, Gq=Gq, calls2=calls2,
                     tiles_g=tiles_g.tolist(), nt1=nt1, nt2=nt2,
                     tmax2=tmax2)
    return per_core, structure, pad_cji, dinv


def _fold_bn(gamma, beta, mean, var, b):
    s = (gamma / np.sqrt(var + EPS)).astype(np.float32)
    t = ((b - mean) * s + beta).astype(np.float32)
    return s.reshape(H, 1), t.reshape(H, 1)


# ---------------------------------------------------------------- bass build
def _build(st_):
    import concourse.bacc as bacc
    import concourse.bass as bass
    import concourse.mybir as mybir
    import concourse.tile as tile

    fp32 = mybir.dt.float32
    fp16 = mybir.dt.float16
    fp8 = mybir.dt.float8e4
    i16 = mybir.dt.int16
    AF = mybir.ActivationFunctionType
    AL = mybir.AluOpType

    G = st_["G"]
    ch_d = st_["ch_d"]
    Gq = st_["Gq"]
    calls2 = st_["calls2"]
    tiles_g = st_["tiles_g"]
    nt1, nt2, TMAX2 = st_["nt1"], st_["nt2"], st_["tmax2"]
    chunk_end = {hi - 1: q for q, (lo, hi) in enumerate(ch_d)}
    g_chunk = {}
    for q, (lo, hi) in enumerate(ch_d):
        for g in range(lo, hi):
            g_chunk[g] = (q, lo)
    starts_t = [0]
    for g in range(1, G):
        starts_t.append(starts_t[-1] + tiles_g[g - 1])
    n_queues = int(os.environ.get("K_QUEUES", "4"))

    nc = bacc.Bacc(None, target_bir_lowering=False, debug=False,
                   num_devices=NC, num_swdge_queues=max(1, n_queues))

    xp_in = nc.dram_tensor("xperm", [128, nt1 * 128], fp16,
                           kind="ExternalInput")
    oh1_in = nc.dram_tensor("oh1", [128, nt1 * 128], fp8,
                            kind="ExternalInput")
    oh2_in = nc.dram_tensor("oh2", [128, nt2 * 128], fp8,
                            kind="ExternalInput")
    w1_in = nc.dram_tensor("W1", [F_IN, H], fp16, kind="ExternalInput")
    w2_in = nc.dram_tensor("W2", [H, H], fp16, kind="ExternalInput")
    wf_in = nc.dram_tensor("Wf", [H, 2], fp16, kind="ExternalInput")
    s1_in = nc.dram_tensor("s1", [H, 1], fp32, kind="ExternalInput")
    t1_in = nc.dram_tensor("t1", [H, 1], fp32, kind="ExternalInput")
    s2_in = nc.dram_tensor("s2", [H, 1], fp32, kind="ExternalInput")
    t2_in = nc.dram_tensor("t2", [H, 1], fp32, kind="ExternalInput")
    bf_in = nc.dram_tensor("bf_rep", [128, 2], fp32, kind="ExternalInput")
    dr_in = nc.dram_tensor("dinvrow", [128, G * 128], fp16,
                           kind="ExternalInput")
    dc_in = nc.dram_tensor("dinvcol", [128, G], fp32, kind="ExternalInput")
    idx2_in = nc.dram_tensor("idx2", [128, nt2 * 8], i16,
                             kind="ExternalInput")
    out_lp = nc.dram_tensor("logp", [128, 2 * G], fp32,
                            kind="ExternalOutput")

    with tile.TileContext(nc) as tc:
        with (
            tc.tile_pool(name="res", bufs=1) as res,
            tc.tile_pool(name="st", bufs=1) as st,
            tc.tile_pool(name="ps", bufs=1, space="PSUM") as ps,
            tc.tile_pool(name="dram", bufs=1, space="DRAM") as dram,
        ):
            w1_t = res.tile([F_IN, H], fp16)
            w2_t = res.tile([H, H], fp16)
            wf_t = res.tile([H, 2], fp16)
            s1_t = res.tile([H, 1], fp32)
            t1_t = res.tile([H, 1], fp32)
            s2_t = res.tile([H, 1], fp32)
            t2_t = res.tile([H, 1], fp32)
            bf_t = res.tile([128, 2], fp32)
            dr_t = res.tile([128, G * 128], fp16)
            dc_t = res.tile([128, G], fp32)
            idx2_t = res.tile([128, nt2 * 8], i16)
            for t_, i_ in ((w1_t, w1_in), (w2_t, w2_in), (wf_t, wf_in),
                           (s1_t, s1_in), (t1_t, t1_in), (s2_t, s2_in),
                           (t2_t, t2_in), (bf_t, bf_in), (dr_t, dr_in),
                           (dc_t, dc_in), (idx2_t, idx2_in)):
                nc.sync.dma_start(out=t_[:], in_=i_[:])

            xs2_shard = [dram.tile([Gq[q] * 128, H], fp16,
                                   name=f"xs2_shard{q}") for q in range(NQ)]
            xs2_full = [dram.tile([NC * Gq[q] * 128, H], fp16,
                                  name=f"xs2_full{q}") for q in range(NQ)]

            lg = res.tile([128, 2 * G], fp32)
            nc.vector.memset(lg[:], 0.0)

            # =================== layer 1: streamed, zero gathers ==========
            def l1_epilogue(g, pre_ap):
                pre_sb = st.tile([128, 128], fp16, name="pre", tag="pre",
                                 bufs=4)
                nc.scalar.copy(out=pre_sb[:], in_=pre_ap)
                h_ps = ps.tile([128, 512], fp32, name="hp", tag="hp", bufs=2)
                nc.tensor.matmul(h_ps[:, :H], w1_t[:], pre_sb[:],
                                 start=True, stop=True)
                hs = st.tile([128, 128], fp16, name="hs", tag="hs", bufs=4)
                nc.vector.tensor_tensor(
                    out=hs[:], in0=h_ps[:, :H],
                    in1=dr_t[:, g * 128:(g + 1) * 128], op=AL.mult)
                hT = st.tile([128, 128], fp16, name="hT", tag="hT", bufs=4)
                nc.scalar.activation(out=hT[:], in_=hs[:], func=AF.Relu,
                                     bias=t1_t[:], scale=s1_t[:])
                x2_ps = ps.tile([128, 512], fp32, name="x2p", tag="x2p",
                                bufs=2)
                nc.tensor.matmul(x2_ps[:, :H], hT[:], w2_t[:],
                                 start=True, stop=True)
                x2_sb = st.tile([128, 128], fp16, name="x2", tag="x2",
                                bufs=4)
                nc.scalar.activation(out=x2_sb[:], in_=x2_ps[:, :H],
                                     func=AF.Copy, bias=0.0,
                                     scale=dc_t[:, g:g + 1])
                q, lo = g_chunk[g]
                nc.sync.dma_start(
                    out=xs2_shard[q][(g - lo) * 128:(g - lo + 1) * 128, :],
                    in_=x2_sb[:])
                if g in chunk_end:
                    qq = chunk_end[g]
                    nc.gpsimd.collective_compute(
                        "AllGather", mybir.AluOpType.bypass,
                        replica_groups=[list(range(NC))],
                        ins=[xs2_shard[qq][:].opt()],
                        outs=[xs2_full[qq][:].opt()],
                    )

            HW = 4  # groups per L1 half-wave (one PSUM bank)
            NHW = (G + HW - 1) // HW
            for hw in range(NHW):
                g_lo, g_hi = hw * HW, min((hw + 1) * HW, G)
                t_lo = starts_t[g_lo]
                t_hi = starts_t[g_hi - 1] + tiles_g[g_hi - 1]
                Tw = t_hi - t_lo
                xpw = st.tile([128, Tw, 128], fp16, name="xpw", tag="xpw",
                              bufs=2)
                nc.sync.dma_start(out=xpw[:],
                                  in_=xp_in[:, t_lo * 128:t_hi * 128])
                ohw = st.tile([128, Tw, 128], fp8, name="ohw", tag="ohw",
                              bufs=2)
                nc.sync.dma_start(out=ohw[:],
                                  in_=oh1_in[:, t_lo * 128:t_hi * 128])
                bank = ps.tile([128, 512], fp32, name="pg", tag="pg", bufs=4)
                tmax_hw = max(tiles_g[g] for g in range(g_lo, g_hi))
                for t in range(tmax_hw):
                    for g in range(g_lo, g_hi):
                        tg = tiles_g[g]
                        if t >= tg:
                            continue
                        col = (g - g_lo) * 128
                        k = starts_t[g] + t - t_lo
                        nc.tensor.matmul(
                            bank[:, col:col + 128], xpw[:, k, :],
                            ohw[:, k, :],
                            start=(t == 0), stop=(t == tg - 1),
                            skip_group_check=True)
                for g in range(g_lo, g_hi):
                    col = (g - g_lo) * 128
                    l1_epilogue(g, bank[:, col:col + 128])

            # =================== layer 2: gathered from xs2_full ==========
            for w, wcalls in enumerate(calls2):
                remaining = {}
                for q, k0, segs in wcalls:
                    for g, tg in segs:
                        remaining[g] = remaining.get(g, 0) + tg
                glist = sorted(remaining)
                g_lo = w * WAVE
                banks2 = [ps.tile([128, 512], fp32, name="pg2", tag="pg",
                                  bufs=4) for _ in range(2)]
                started = set()

                def pg2_ap(g):
                    b = banks2[(g - g_lo) // 4]
                    col = ((g - g_lo) % 4) * 128
                    return b[:, col:col + 128]

                for q, k0, segs in wcalls:
                    tcall = sum(tg for _, tg in segs)
                    ni = tcall * 128
                    msg = st.tile([128, TMAX2, 128], fp16, name="msg",
                                  tag="msg", bufs=3)
                    src_ap = xs2_full[q][:]
                    nc.gpsimd.dma_gather(
                        msg[:, :tcall, :], src_ap,
                        idx2_t[:, k0 * 8:(k0 + tcall) * 8],
                        ni, ni, H, elem_step=src_ap.ap[0][0],
                        single_packet=False,
                        queue_num=(q % n_queues) if n_queues > 1 else 0)
                    oh2w = st.tile([128, TMAX2, 128], fp8, name="oh2w",
                                   tag="oh2w", bufs=3)
                    nc.sync.dma_start(
                        out=oh2w[:, :tcall, :],
                        in_=oh2_in[:, k0 * 128:(k0 + tcall) * 128])
                    order = []
                    tl = 0
                    for g, tg in segs:
                        for j in range(tg):
                            order.append((j, g, tl))
                            tl += 1
                    order.sort(key=lambda x: (x[0], x[1]))
                    for _, g, tl in order:
                        first = g not in started
                        started.add(g)
                        nc.tensor.matmul(
                            pg2_ap(g), msg[:, tl, :], oh2w[:, tl, :],
                            start=first, stop=(remaining[g] == 1),
                            skip_group_check=True)
                        remaining[g] -= 1
                for g in glist:
                    hs2 = st.tile([128, 128], fp16, name="hs2", tag="hs",
                                  bufs=4)
                    nc.vector.tensor_tensor(
                        out=hs2[:], in0=pg2_ap(g),
                        in1=dr_t[:, g * 128:(g + 1) * 128], op=AL.mult)
                    hT2 = st.tile([128, 128], fp16, name="hT2", tag="hT",
                                  bufs=4)
                    nc.scalar.activation(out=hT2[:], in_=hs2[:],
                                         func=AF.Relu, bias=t2_t[:],
                                         scale=s2_t[:])
                    plg = ps.tile([128, 512], fp32, name="plg", tag="hp",
                                  bufs=2)
                    nc.tensor.matmul(plg[:, 0:2], hT2[:], wf_t[:],
                                     start=True, stop=True)
                    nc.vector.tensor_add(out=lg[:, 2 * g:2 * g + 2],
                                         in0=plg[:, 0:2], in1=bf_t[:])

            # =================== log-softmax over the 2 logits ============
            def strided(base, start):
                a = base[:]
                return bass.AP(a.tensor, a.offset + start, [a.ap[0], [2, G]])

            z0, z1 = strided(lg, 0), strided(lg, 1)
            mx = res.tile([128, G], fp32)
            nc.vector.tensor_tensor(out=mx[:], in0=z0, in1=z1, op=AL.max)
            sm0 = res.tile([128, G], fp32)
            sm1 = res.tile([128, G], fp32)
            nc.vector.tensor_sub(out=sm0[:], in0=z0, in1=mx[:])
            nc.vector.tensor_sub(out=sm1[:], in0=z1, in1=mx[:])
            e0 = res.tile([128, G], fp32)
            e1 = res.tile([128, G], fp32)
            nc.scalar.activation(out=e0[:], in_=sm0[:], func=AF.Exp)
            nc.scalar.activation(out=e1[:], in_=sm1[:], func=AF.Exp)
            se = res.tile([128, G], fp32)
            nc.vector.tensor_add(out=se[:], in0=e0[:], in1=e1[:])
            ls = res.tile([128, G], fp32)
            nc.scalar.activation(out=ls[:], in_=se[:], func=AF.Ln)
            nc.vector.tensor_sub(out=sm0[:], in0=sm0[:], in1=ls[:])
            nc.vector.tensor_sub(out=sm1[:], in0=sm1[:], in1=ls[:])
            lpo = res.tile([128, 2 * G], fp32)
            nc.vector.tensor_copy(out=strided(lpo, 0), in_=sm0[:])
            nc.vector.tensor_copy(out=strided(lpo, 1), in_=sm1[:])
            nc.sync.dma_start(out=out_lp[:], in_=lpo[:])

    nc.compile()
    return nc


# ---------------------------------------------------------------- main entry
def _run(x, edge_index, game_indices,
         W1, b1, g1, be1, m1, v1, W2, b2, g2, be2, m2, v2, Wf, bf,
         trace=False):
    from concourse import bass_utils

    ei = np.asarray(edge_index)
    key = ("prep", int(ei[0, 0]), int(ei.sum() % (1 << 31)))
    if key in _CACHE:
        per_core, structure, pad_cji, dinv = _CACHE[key]
    else:
        per_core, structure, pad_cji, dinv = _prepare(ei)
        _CACHE.clear()
        _CACHE[key] = (per_core, structure, pad_cji, dinv)

    skey = ("bass", structure["G"], structure["nt1"], structure["nt2"],
            structure["tmax2"])
    if skey in _CACHE:
        nc = _CACHE[skey]
    else:
        nc = _build(structure)
        _CACHE[skey] = nc

    nt1 = structure["nt1"]

    # xs = x * dinv[src]: the src-side norm folded into the table rows
    xs = (np.asarray(x, dtype=np.float32)
          * dinv.astype(np.float32)[:, None]).astype(np.float16)
    s1, t1 = _fold_bn(np.asarray(g1), np.asarray(be1), np.asarray(m1),
                      np.asarray(v1), np.asarray(b1))
    s2, t2 = _fold_bn(np.asarray(g2), np.asarray(be2), np.asarray(m2),
                      np.asarray(v2), np.asarray(b2))
    bf_rep = np.broadcast_to(np.asarray(bf, dtype=np.float32),
                             (128, 2)).copy()
    w1h = np.asarray(W1, np.float16)
    w2h = np.asarray(W2, np.float16)
    wfh = np.asarray(Wf, np.float16)

    import ml_dtypes
    in_maps = []
    for c in range(NC):
        pc = per_core[c]
        xp_idx = pc["xp_idx"]
        xp = xs[np.maximum(xp_idx, 0)]
        xp[xp_idx < 0] = 0
        xpt = np.ascontiguousarray(
            xp.reshape(nt1, 128, F_IN).transpose(1, 0, 2)
        ).reshape(128, nt1 * F_IN)
        in_maps.append(dict(
            xperm=xpt,
            oh1=pc["oh1"].view(ml_dtypes.float8_e4m3),
            oh2=pc["oh2"].view(ml_dtypes.float8_e4m3),
            W1=w1h, W2=w2h, Wf=wfh, s1=s1, t1=t1, s2=s2, t2=t2,
            bf_rep=bf_rep, dinvrow=pc["dinvrow"], dinvcol=pc["dinvcol"],
            idx2=pc["idx2"],
        ))
    res = bass_utils.run_bass_kernel_spmd(
        nc, in_maps, core_ids=list(range(NC)), trace=trace)

    gi = np.asarray(game_indices, dtype=np.int64)
    cji = pad_cji[gi]
    lp = np.stack([res.results[c]["logp"] for c in range(NC)])
    out = np.empty((gi.shape[0], 2), dtype=np.float32)
    out[:, 0] = lp[cji[:, 0], cji[:, 2], 2 * cji[:, 1]]
    out[:, 1] = lp[cji[:, 0], cji[:, 2], 2 * cji[:, 1] + 1]
    return out, res


def kernel(**inputs):
    out, _ = _run(**inputs)
    return out


def kernel_profiled(**inputs):
    out, res = _run(**inputs, trace=True)
    return out, res
